# revision 14
# baseline (speedup 1.0000x reference)
"""Llama GQA attention (B=2, T=2048, C=2048, 32 Q heads / 8 KV heads, D=64,
interleaved RoPE, causal) on 8 TRN2 NeuronCores.

Sharding: core c -> (batch b = c//4, head-group g = c%4). Each core handles one
batch element and 8 Q heads / 2 KV heads (wq/wk/wv column shards, wo row
shard), producing a partial y[b]; the host sums the 4 partials per batch.

v2 (vs the f32r baseline):
- bf16 operands on the PE everywhere (1 cyc/streamed-col, cheap FWL weight
  loads; host pre-casts inputs, E/O rope permutation folded into wq/wk cols).
- head-pair packed QK: heads h and h+4 live on PE rows 0-63 / 64-127 (K=64
  each) and run concurrently via base-partition row groups.
- exp batched over [128, 2, 512] two-bank PSUM groups on ACT.
- full-width RoPE: 4 cross-partition swap copies (gpsimd) + 2 muls with
  [c;c;c;c] / [-s;s;-s;s] tables + per-head adds on DVE.
- softmax denominators via the ones-column AV trick + reciprocal_approx_fast
  (the DVE iterative reciprocal cost 3.4us per [1,512] row in the baseline).
- software pipelining: o_proj chains of t-block tb-1 are interleaved between
  attention groups of tb so the PE stays dense while ACT chews exps (HAM
  p-state needs >3us continuous PE activity to reach 2.4 GHz).
"""
import sys

sys.path.insert(0, "/opt/trn_rl_repo")
sys.path.insert(0, "/opt/trn_rl_repo/concourse")

import numpy as np

import concourse.bass as bass
import concourse.tile as tile
from concourse import bacc, mybir
from concourse.bass_utils import run_bass_kernel_spmd
from concourse.masks import make_identity

T = 2048
C = 2048
D = 64
HQ = 8          # q heads per core
HKV = 2         # kv heads per core
QCOLS = HQ * D  # 512
KCOLS = HKV * D  # 128
TB = 512        # t-block (i-block) size
NTB = T // TB   # 4
CCH = C // 128  # 16 c-chunks
SCALE = 0.125   # 1/sqrt(64)

f32 = mybir.dt.float32
bf16 = mybir.dt.bfloat16
EXPF = mybir.ActivationFunctionType.Exp

_cache = {}


def _build_program():
    nc = bacc.Bacc("TRN2", target_bir_lowering=False, debug=False, num_devices=1)
    d = {}
    d["xt"] = nc.dram_tensor("xt", [C, T], bf16, kind="ExternalInput").ap()
    d["wq"] = nc.dram_tensor("wq", [C, QCOLS], bf16, kind="ExternalInput").ap()
    d["wk"] = nc.dram_tensor("wk", [C, KCOLS], bf16, kind="ExternalInput").ap()
    d["wv"] = nc.dram_tensor("wv", [C, KCOLS], bf16, kind="ExternalInput").ap()
    d["wo"] = nc.dram_tensor("wo", [QCOLS, C], bf16, kind="ExternalInput").ap()
    d["ropec"] = nc.dram_tensor("ropec", [128, T], bf16, kind="ExternalInput").ap()
    d["ropes"] = nc.dram_tensor("ropes", [128, T], bf16, kind="ExternalInput").ap()
    d["msk"] = nc.dram_tensor("msk", [4, 128, TB], bf16, kind="ExternalInput").ap()
    y_d = nc.dram_tensor("y", [T, C], bf16, kind="ExternalOutput").ap()

    with tile.TileContext(nc) as tc:
        _emit(nc, tc, d, y_d)
    nc.compile()
    return nc


def _emit(nc, tc, d, y_d):
    from contextlib import ExitStack

    ctx = ExitStack()
    with ctx:
        sb_w = ctx.enter_context(tc.tile_pool(name="sb_w", bufs=1))
        sb = ctx.enter_context(tc.tile_pool(name="sb", bufs=1))
        sb_p = ctx.enter_context(tc.tile_pool(name="sb_p", bufs=3))
        sb_t = ctx.enter_context(tc.tile_pool(name="sb_t", bufs=2))
        sb_y = ctx.enter_context(tc.tile_pool(name="sb_y", bufs=2))
        ps_a = ctx.enter_context(tc.tile_pool(name="ps_a", bufs=2, space="PSUM"))
        ps_s = ctx.enter_context(tc.tile_pool(name="ps_s", bufs=1, space="PSUM"))
        ps_o = ctx.enter_context(tc.tile_pool(name="ps_o", bufs=2, space="PSUM"))

        # ---- weights (issued below, after the first x^T block) ----
        wq_sb = sb_w.tile([128, CCH, QCOLS], bf16)
        wk_sb = sb_w.tile([128, CCH, KCOLS], bf16)
        wv_sb = sb_w.tile([128, CCH, KCOLS], bf16)
        wo_sb = sb_w.tile([128, 4, C], bf16)

        # ---- constants ----
        ident_f = sb.tile([128, 128], f32)
        make_identity(nc, ident_f[:])
        ident = sb.tile([128, 128], bf16)
        nc.gpsimd.tensor_copy(ident[:], ident_f[:])
        ropec = sb.tile([128, T], bf16)
        ropes = sb.tile([128, T], bf16)
        nc.sync.dma_start(ropec[:], d["ropec"][:, :])
        nc.sync.dma_start(ropes[:], d["ropes"][:, :])
        masks = []
        for mi in range(4):
            m = sb.tile([128, TB], bf16, tag=f"mask{mi}")
            nc.sync.dma_start(m[:], d["msk"][mi, :, :])
            masks.append(m)

        # ---- persistent state (xT/qt/ot double-buffered for cross-tb overlap)
        xT_tiles = [sb.tile([128, CCH, TB], bf16, tag=f"xT{i}", name=f"xT{i}")
                    for i in range(2)]
        qt_tiles = [sb.tile([128, 4, TB], bf16, tag=f"qt{i}", name=f"qt{i}")
                    for i in range(2)]
        ot_tiles = [sb.tile([128, 4, TB], bf16, tag=f"ot{i}", name=f"ot{i}")
                    for i in range(2)]
        kt_tiles = [sb.tile([128, TB], bf16, tag=f"kt{i}", name=f"kt{i}")
                    for i in range(NTB)]
        v_tiles = [sb.tile([128, 2, D + 1], bf16, tag=f"v{i}", name=f"v{i}")
                   for i in range(NTB * 4)]
        for i in range(NTB * 4):
            nc.gpsimd.memset(v_tiles[i][:, 0, D:D + 1], 1.0)
            nc.gpsimd.memset(v_tiles[i][:, 1, D:D + 1], 1.0)

        # prefetch x^T for tb=0 (host supplies x pre-transposed); interleave
        # the weight loads in first-use order across the sync + gpsimd queues
        # so the first projection chain can start ~15us in.
        for cc in range(CCH):
            nc.sync.dma_start(xT_tiles[0][:, cc, :],
                              d["xt"][cc * 128:(cc + 1) * 128, 0:TB])
            nc.gpsimd.dma_start(wq_sb[:, cc, :],
                                d["wq"][cc * 128:(cc + 1) * 128, :])
        for cc in range(CCH):
            eng = nc.sync if cc % 2 == 0 else nc.gpsimd
            eng.dma_start(wk_sb[:, cc, :], d["wk"][cc * 128:(cc + 1) * 128, :])
            eng2 = nc.gpsimd if cc % 2 == 0 else nc.sync
            eng2.dma_start(wv_sb[:, cc, :], d["wv"][cc * 128:(cc + 1) * 128, :])
        for oc in range(4):
            nc.gpsimd.dma_start(wo_sb[:, oc, :],
                                d["wo"][oc * 128:(oc + 1) * 128, :])

        def emit_phase_a(tb):
            pass

        def emit_rope(psum, tb, dests):
            """RoPE a full [128, TB] projection tile in PSUM.

            psum rows are [E0(32), O0(32), E1(32), O1(32)] for two heads; the
            roped result is psum*ropec + swap32(psum)*ropes with ropec rows
            [c;c;c;c] and ropes rows [-s;s;-s;s]. dests = list of
            (psum_row_base, nrows, out_ap) slices to write."""
            i0 = tb * TB
            c1 = sb_t.tile([128, TB], bf16, tag="ropec1")
            nc.vector.tensor_copy(c1[:], psum[:])  # one PSUM drain (DVE)
            sw = sb_t.tile([128, TB], bf16, tag="ropesw")
            for g, srcg in ((0, 1), (1, 0), (2, 3), (3, 2)):
                nc.vector.tensor_copy(sw[g * 32:(g + 1) * 32, :],
                                      c1[srcg * 32:(srcg + 1) * 32, :])
            t1 = sb_t.tile([128, TB], bf16, tag="ropet1")
            t2 = sb_t.tile([128, TB], bf16, tag="ropet2")
            nc.vector.tensor_mul(t1[:], c1[:], ropec[:, i0:i0 + TB])
            nc.vector.tensor_mul(t2[:], sw[:], ropes[:, i0:i0 + TB])
            for base, nrows, out_ap in dests:
                nc.vector.tensor_add(out_ap, t1[base:base + nrows, :],
                                     t2[base:base + nrows, :])

        def emit_phase_b(tb):
            """Projections + rope + V prep; also prefetch next x block."""
            xT = xT_tiles[tb % 2]
            qt_sb = qt_tiles[tb % 2]
            if tb + 1 < NTB:
                ni0 = (tb + 1) * TB
                for cc in range(CCH):
                    nc.sync.dma_start(
                        xT_tiles[(tb + 1) % 2][:, cc, :],
                        d["xt"][cc * 128:(cc + 1) * 128, ni0:ni0 + TB])

            for j in range(4):  # Q^T in 128-row chunks: heads 2j, 2j+1
                pq = ps_a.tile([128, TB], f32, tag="pa", name="pq")
                for cc in range(CCH):
                    nc.tensor.matmul(pq[:], wq_sb[:, cc, j * 128:(j + 1) * 128],
                                     xT[:, cc, :], start=(cc == 0),
                                     stop=(cc == CCH - 1))
                h0, h1 = 2 * j, 2 * j + 1
                emit_rope(pq[:], tb, [
                    (0, 64, qt_sb[(h0 // 4) * 64:(h0 // 4) * 64 + 64, h0 % 4, :]),
                    (64, 64, qt_sb[(h1 // 4) * 64:(h1 // 4) * 64 + 64, h1 % 4, :]),
                ])

            pk = ps_a.tile([128, TB], f32, tag="pa", name="pk")
            for cc in range(CCH):
                nc.tensor.matmul(pk[:], wk_sb[:, cc, :], xT[:, cc, :],
                                 start=(cc == 0), stop=(cc == CCH - 1))
            emit_rope(pk[:], tb, [(0, 128, kt_tiles[tb][:, :])])

            pv = ps_a.tile([128, TB], f32, tag="pa", name="pv")
            for cc in range(CCH):
                nc.tensor.matmul(pv[:], wv_sb[:, cc, :], xT[:, cc, :],
                                 start=(cc == 0), stop=(cc == CCH - 1))
            vt_tmp = sb_t.tile([128, TB], bf16, tag="vt_tmp", bufs=1)
            nc.vector.tensor_copy(vt_tmp[:], pv[:])
            for j2 in range(4):
                pvt = ps_a.tile([128, KCOLS], bf16, tag="pa", name="pvt")
                nc.tensor.transpose(pvt[:], vt_tmp[:, j2 * 128:(j2 + 1) * 128],
                                    ident[:])
                vt = v_tiles[tb * 4 + j2]
                nc.vector.tensor_copy(vt[:, 0, 0:D], pvt[:, 0:D])
                nc.vector.tensor_copy(vt[:, 1, 0:D], pvt[:, D:2 * D])

        def emit_oproj_chains(tb):
            """Returns a list of thunks, one per (t2) group of o_proj work for
            t-block tb (4 matmul chains + 1 y DMA each)."""
            ot_sb = ot_tiles[tb % 2]
            i0 = tb * TB
            thunks = []
            for t2 in range(4):
                def chain(t2=t2):
                    y_sb = sb_y.tile([128, 4, TB], bf16, tag="ysb")
                    for cbp in range(2):
                        pys = [ps_a.tile([128, TB], f32, tag="pa", name="py")
                               for _ in range(2)]
                        for oc in range(4):
                            for jj in range(2):
                                cb = 2 * cbp + jj
                                nc.tensor.matmul(
                                    pys[jj][:],
                                    ot_sb[:, oc, t2 * 128:(t2 + 1) * 128],
                                    wo_sb[:, oc, cb * TB:(cb + 1) * TB],
                                    start=(oc == 0), stop=(oc == 3))
                        for jj in range(2):
                            nc.vector.tensor_copy(y_sb[:, 2 * cbp + jj, :],
                                                  pys[jj][:])
                    nc.sync.dma_start(
                        y_d[i0 + t2 * 128:i0 + (t2 + 1) * 128, :],
                        y_sb[:, :, :])
                thunks.append(chain)
            return thunks

        def emit_attention(tb, fill):
            """Attention for t-block tb; pops thunks from `fill` between
            groups to keep the PE busy while ACT runs exps."""
            qt_sb = qt_tiles[tb % 2]
            ot_sb = ot_tiles[tb % 2]
            njt = 4 * (tb + 1)
            for p in range(4):
                ha, hb = p, p + 4
                po_a = ps_o.tile([128, TB], f32, tag="po", name="po_a")
                po_b = ps_o.tile([128, TB], f32, tag="po", name="po_b")
                prev = None
                for jt0 in range(0, njt, 2):
                    jts = (jt0, jt0 + 1)
                    ps_ga = ps_s.tile([128, 2, TB], f32, tag="sa", name="ps_ga")
                    ps_gb = ps_s.tile([128, 2, TB], f32, tag="sb", name="ps_gb")
                    for ii, jt in enumerate(jts):
                        # diagonal blocks: only tq-chunks >= jt-4tb are live
                        mi = jt - 4 * tb
                        f0 = max(0, mi) * 128
                        ksa = kt_tiles[jt // 4][0:64,
                                                (jt % 4) * 128:(jt % 4) * 128 + 128]
                        nc.tensor.matmul(ps_ga[:, ii, f0:], ksa,
                                         qt_sb[0:64, p, f0:], start=True,
                                         stop=True, tile_position=(0, 0))
                        ksb = kt_tiles[jt // 4][64:128,
                                                (jt % 4) * 128:(jt % 4) * 128 + 128]
                        nc.tensor.matmul(ps_gb[:, ii, f0:], ksb,
                                         qt_sb[64:128, p, f0:], start=True,
                                         stop=True, tile_position=(64, 0))
                    p_a = sb_p.tile([128, 2, TB], bf16, tag="p_a")
                    nc.scalar.activation(p_a[:, :, :], ps_ga[:, :, :], EXPF,
                                         bias=0.0, scale=SCALE)
                    p_b = sb_p.tile([128, 2, TB], bf16, tag="p_b")
                    nc.scalar.activation(p_b[:, :, :], ps_gb[:, :, :], EXPF,
                                         bias=0.0, scale=SCALE)
                    for ii, jt in enumerate(jts):
                        mi = jt - 4 * tb
                        if 0 <= mi <= 3:
                            nc.vector.tensor_mul(p_a[:, ii, :], p_a[:, ii, :],
                                                 masks[mi][:])
                            nc.vector.tensor_mul(p_b[:, ii, :], p_b[:, ii, :],
                                                 masks[mi][:])
                    # AV for the previous group was emitted already; emit this
                    # group's AV now (QK of the next group will overlap exp).
                    for ii, jt in enumerate(jts):
                        mi = jt - 4 * tb
                        f0 = max(0, mi) * 128
                        nc.tensor.matmul(po_a[0:D + 1, f0:], v_tiles[jt][:, 0, :],
                                         p_a[:, ii, f0:], start=(jt == 0),
                                         stop=(jt == njt - 1))
                    for ii, jt in enumerate(jts):
                        mi = jt - 4 * tb
                        f0 = max(0, mi) * 128
                        nc.tensor.matmul(po_b[0:D + 1, f0:], v_tiles[jt][:, 1, :],
                                         p_b[:, ii, f0:], start=(jt == 0),
                                         stop=(jt == njt - 1))
                    if fill:
                        fill.pop(0)()
                # normalize: ones-column denominators live in row D
                for po, h in ((po_a, ha), (po_b, hb)):
                    # custom-DVE ops misread nonzero base partitions: stage the
                    # denominator row to partition 0 in SBUF first.
                    den = sb_t.tile([1, TB], f32, tag="den", bufs=2)
                    nc.vector.tensor_copy(den[:], po[D:D + 1, :])
                    r = sb_t.tile([1, TB], f32, tag="r_row", bufs=2)
                    nc.vector.reciprocal_approx_fast(r[:], den[:])
                    rb = sb_t.tile([D, TB], f32, tag="rb", bufs=2)
                    nc.gpsimd.partition_broadcast(rb[:], r[:])
                    nc.vector.tensor_mul(
                        ot_sb[(h % 2) * 64:(h % 2) * 64 + 64, h // 2, :],
                        po[0:D, :], rb[:])

        # ---- main schedule ----
        pending = []
        for tb in range(NTB):
            emit_phase_a(tb)
            emit_phase_b(tb)
            emit_attention(tb, pending)
            pending = emit_oproj_chains(tb)
        for t in pending:
            t()

        dbg = globals().get("_DBG")
        if dbg:
            lb = (NTB - 1) % 2
            nc.sync.dma_start(dbg["xT"], xT_tiles[lb][:, :, :])
            nc.sync.dma_start(dbg["qt"], qt_tiles[lb][:, :, :])
            nc.sync.dma_start(dbg["kt0"], kt_tiles[0][:, :])
            nc.sync.dma_start(dbg["vt0"], v_tiles[0][:, :, :])
            nc.sync.dma_start(dbg["ot"], ot_tiles[lb][:, :, :])


def _perm_cols(w):
    """Reorder each 64-wide head block's columns to [evens, odds]."""
    cols = []
    for h0 in range(0, w.shape[1], D):
        cols.extend(range(h0, h0 + D, 2))
        cols.extend(range(h0 + 1, h0 + D, 2))
    return np.ascontiguousarray(w[:, cols])


def _host_tables():
    inv = (1.0 / (10000.0 ** (np.arange(0, D, 2) / D)))
    ang = np.arange(T)[None, :] * inv[:, None]          # [32, T]
    cos = np.cos(ang).astype(np.float32)
    sin = np.sin(ang).astype(np.float32)
    ropec = np.tile(cos, (4, 1))                        # [128, T] = [c;c;c;c]
    ropes = np.concatenate([-sin, sin, -sin, sin], axis=0)  # [-s;s;-s;s]
    msk = np.zeros((4, 128, TB), dtype=np.float32)
    for mi, off in enumerate((0, -128, -256, -384)):
        p = np.arange(128)[:, None]
        f = np.arange(TB)[None, :]
        msk[mi] = (off + f - p >= 0).astype(np.float32)
    return ropec, ropes, msk


def kernel(x, wq, wk, wv, wo, _trace=False):
    import ml_dtypes
    bf = ml_dtypes.bfloat16

    if "nc" not in _cache:
        _cache["nc"] = _build_program()
    nc = _cache["nc"]

    ropec, ropes, msk = _host_tables()
    msk_bf = msk.astype(bf)
    in_maps = []
    for c in range(8):
        b, g = c // 4, c % 4
        in_maps.append({
            "xt": np.ascontiguousarray(x[b].T).astype(bf),
            "wq": _perm_cols(wq[:, g * QCOLS:(g + 1) * QCOLS]).astype(bf),
            "wk": _perm_cols(wk[:, g * KCOLS:(g + 1) * KCOLS]).astype(bf),
            "wv": np.ascontiguousarray(wv[:, g * KCOLS:(g + 1) * KCOLS]).astype(bf),
            "wo": np.ascontiguousarray(wo[g * QCOLS:(g + 1) * QCOLS, :]).astype(bf),
            "ropec": ropec.astype(bf), "ropes": ropes.astype(bf), "msk": msk_bf,
        })

    res = run_bass_kernel_spmd(nc, in_maps, core_ids=list(range(8)),
                               trace=_trace)
    _cache["last_res"] = res
    y = np.zeros((2, T, C), dtype=np.float32)
    for c in range(8):
        y[c // 4] += res.results[c]["y"]
    return y
